# revision 59
# baseline (speedup 1.0000x reference)
"""ChebConv layer (B=128, N=512, F=32, K=3) on 8 TRN2 NeuronCores.

Math: with lambda_max = 2.0, Lhat = -Ahat, Ahat = S A S with S = diag(dinv).
Host folds S into the adjacency: ship adjF = 256*Ahat as fp8 (entries ~A,
good e4m3 range) and xq = 16*x as fp8. Then on-chip:
    uT  = (16x)^T (256Ahat)^T          = 4096*y1^T,  y1 = Ahat x  (T1 = -y1)
    y1T = uT/16 (bf16) = 256*y1; transpose -> y1n (fp8 natural layout)
    vT  = y1n^T (256Ahat)^T            = 65536*v^T,  v = Ahat^2 x
    out = relu( x(W0-W2) + (256y1)(-W1/256) + (256v)(2W2/256) + b ) + x
No per-node dinv scaling on-chip at all - it all lives in the host prep.

Sharding: data-parallel over batch, 16 samples/core as 4 groups of 4.
Per group of 4 samples (quadrant q = partition group 32q:32q+32):
  - u/v matmuls use fp8 DoubleRow perf mode: chunk pairs (c0,c1),(c2,c3)
    fused into one instruction each (0.5 cyc/row = 4x fewer PE column
    cycles vs one matmul per 128-chunk), 4 col groups concurrent via
    tile_position=(0,32q).
  - epilogue terms t0 (x*W0') and t1 (y1*W1') issue right after y1T so
    only the t2 (v) matmul trails the v-stage.
DMA: adj chunks spread over the three DGE paths (SP ring, ACT ring,
SWDGE) so each group's DoubleRow pair lands near-simultaneously; xq/xt
ride the ACT ring up front, outs ride the SP ring.
"""

import os
import sys

sys.path.insert(0, "/opt/trn_rl_repo")

import numpy as np

import concourse.bass as bass
from concourse import bacc
import concourse.mybir as mybir
import concourse.tile as tile
from concourse.bass_utils import run_bass_kernel_spmd
from contextlib import ExitStack

B, N, F = 128, 512, 32
NCORES = 8
S = B // NCORES          # samples per core (16)
P = 128                  # SBUF partitions
C = N // P               # m-chunks per sample (4)
Q = 4                    # samples per group (one per quadrant)
G = S // Q               # groups per core (4)

f32 = mybir.dt.float32
bf16 = mybir.dt.bfloat16
f8 = mybir.dt.float8e4

_cache = {}


def _install_ntff_hook():
    """Provide antenv.axon_hooks (missing in this image) so trace=True works."""
    import contextlib
    import ctypes
    import types

    try:
        from antenv.axon_hooks import get_axon_ntff_profile_hook  # noqa: F401
        return
    except ImportError:
        pass
    so_path = "/opt/axon/libaxon_pjrt.so"
    if not os.path.exists(so_path):
        return
    lib = ctypes.CDLL(so_path)
    if not hasattr(lib, "axon_start_nrt_profile"):
        return
    lib.axon_start_nrt_profile.argtypes = [
        ctypes.POINTER(ctypes.c_int64), ctypes.c_size_t,
    ]
    lib.axon_start_nrt_profile.restype = ctypes.c_int64
    lib.axon_stop_nrt_profile.argtypes = [ctypes.c_char_p]
    lib.axon_stop_nrt_profile.restype = ctypes.c_int64

    @contextlib.contextmanager
    def _hook(output_dir, device_ids):
        import jax

        jax.devices()
        if device_ids:
            ids = (ctypes.c_int64 * len(device_ids))(*device_ids)
            rc = lib.axon_start_nrt_profile(ids, len(device_ids))
        else:
            rc = lib.axon_start_nrt_profile(None, 0)
        if rc != 0:
            raise RuntimeError(f"axon_start_nrt_profile rc={rc}")
        try:
            yield
        finally:
            n = lib.axon_stop_nrt_profile(str(output_dir).encode())
            print(f"profile: {n} file(s) written to {output_dir}", file=sys.stderr)

    mod = types.ModuleType("antenv.axon_hooks")
    state = {"hook": _hook}
    mod.get_axon_ntff_profile_hook = lambda: state["hook"]
    mod.set_axon_ntff_profile_hook = lambda h: state.update(hook=h)
    sys.modules["antenv.axon_hooks"] = mod


def build_nc(zero_bias):
    nc = bacc.Bacc()
    adj_d = nc.declare_dram_parameter("adj8", [P, G, C, Q, N], f8, isOutput=False)
    xq_d = nc.declare_dram_parameter("xq8", [P, G, C, Q, F], f8, isOutput=False)
    xt_d = nc.declare_dram_parameter("xt", [P, G, N], bf16, isOutput=False)
    wept_d = nc.declare_dram_parameter("wept", [P, 3, F], bf16, isOutput=False)
    id_d = nc.declare_dram_parameter("ident", [P, P], bf16, isOutput=False)
    b_d = nc.declare_dram_parameter("bcol", [P, 1], f32, isOutput=False)
    out_d = nc.declare_dram_parameter("out", [G, P, N], bf16, isOutput=True)

    Copy = mybir.ActivationFunctionType.Copy
    Relu = mybir.ActivationFunctionType.Relu

    with tile.TileContext(nc) as tc, ExitStack() as ctx:
        consts = ctx.enter_context(tc.tile_pool(name="consts", bufs=1))
        adj_pool = ctx.enter_context(tc.tile_pool(name="adj", bufs=G))
        work = ctx.enter_context(tc.tile_pool(name="work", bufs=3))
        ps_u = ctx.enter_context(tc.tile_pool(name="psu", bufs=1, space="PSUM"))
        ps_v = ctx.enter_context(tc.tile_pool(name="psv", bufs=1, space="PSUM"))
        ps_a = ctx.enter_context(tc.tile_pool(name="psa", bufs=2, space="PSUM"))
        ps_t = ctx.enter_context(tc.tile_pool(name="pst", bufs=2, space="PSUM"))
        ps_w = ctx.enter_context(tc.tile_pool(name="psw", bufs=1, space="PSUM"))

        ats = [adj_pool.tile([P, C, Q, N], f8, tag=f"at{g}", name=f"at{g}")
               for g in range(G)]
        xq = consts.tile([P, G, C, Q, F], f8, tag="xq")
        xt = consts.tile([P, G, N], bf16, tag="xt")
        ident = consts.tile([P, P], bf16, tag="ident")
        wept = consts.tile([P, 3, F], bf16, tag="wept")
        bcol = consts.tile([P, 1], f32, tag="bcol")

        # PE warm-up: dependency-free matmuls bridge from engine start to
        # the first adj pair arrival so HAM hits K=8/8 as real work begins
        # (a cold PE runs at reduced duty and falls behind the stream).
        # PE warm-up: dependency-free matmuls bridge from engine start
        # (~7us) to the first adj pair arrival (~13us). HAM holds the PE
        # at full speed (k=8) only while it is continuously busy - an
        # idle gap triggers a ~10us half-speed (k=4) penalty - so the
        # warm-up must hand off seamlessly to real work.
        warm = consts.tile([P, N], bf16, tag="warm")
        nc.vector.memset(warm, 0.0)
        wps = ps_w.tile([F, N], f32, tag="wps")
        for _ in range(24):
            nc.tensor.matmul(wps, warm[:, 0:F], warm, start=True, stop=True)

        def fillers(n):
            """Junk matmuls that keep the PE continuously busy across an
            expected DMA-arrival gap (an idle PE drops to HAM k=4)."""
            for _ in range(n):
                nc.tensor.matmul(wps[:, 0:128], warm[:, 0:F], warm[:, 0:128],
                                 start=True, stop=True)

        # --- input DMAs, 512KB chunk-pairs. Per group, (c0,c1) rides the
        # SP ring and (c2,c3) the ACT ring in lockstep (~3.3us cadence);
        # xq leads the SP ring so u(g0) can start on first-pair arrival.
        # Consts + xt ride the SWDGE (latency-insensitive). Outs join the
        # SP ring from stage_e.
        nc.sync.dma_start(out=xq, in_=xq_d[:, :, :, :, :])
        nc.gpsimd.dma_start(out=ident, in_=id_d[:, :])
        nc.gpsimd.dma_start(out=xt[:, 0:2, :], in_=xt_d[:, 0:2, :])
        nc.gpsimd.dma_start(out=wept, in_=wept_d[:, :, :])
        nc.gpsimd.dma_start(out=bcol, in_=b_d[:, :])
        nc.scalar.dma_start(out=ats[0][:, 2:4], in_=adj_d[:, 0, 2:4])
        nc.sync.dma_start(out=ats[0][:, 0], in_=adj_d[:, 0, 0])
        nc.sync.dma_start(out=ats[0][:, 1], in_=adj_d[:, 0, 1])
        nc.scalar.dma_start(out=ats[1][:, 2:4], in_=adj_d[:, 1, 2:4])
        nc.sync.dma_start(out=ats[1][:, 0:2], in_=adj_d[:, 1, 0:2])
        nc.gpsimd.dma_start(out=ats[2][:, 2:4], in_=adj_d[:, 2, 2:4])
        nc.sync.dma_start(out=ats[2][:, 0:2], in_=adj_d[:, 2, 0:2])
        nc.scalar.dma_start(out=ats[3][:, 2:4], in_=adj_d[:, 3, 2:4])
        nc.sync.dma_start(out=ats[3][:, 0:2], in_=adj_d[:, 3, 0:2])
        nc.gpsimd.dma_start(out=xt[:, 2:4, :], in_=xt_d[:, 2:4, :])

        def stage_u(g, half, uT=None):
            """uT = 4096*y1^T; 4 col groups stream the PE concurrently.
            Issued as two chunk-pair halves matching DMA arrival."""
            at = ats[g]
            if uT is None:
                uT = ps_u.tile([P, N], f32, tag="uT")
            for c in (0, 1) if half == 0 else (2, 3):
                for q in range(Q):
                    nc.tensor.matmul(
                        uT[32 * q:32 * q + 32, :],
                        xq[:, g, c, q, :],
                        at[:, c, q, :],
                        start=(c == 0), stop=(c == C - 1),
                        tile_position=(0, 32 * q),
                    )
            return uT

        def stage_m(g, uT):
            """y1T = uT/16 (bf16), transpose to y1n fp8; early epilogue
            terms t0 (x) and t1 (y1) into acc. y1T is computed as two
            column halves on DVE and ACT in parallel, and y1n as two
            chunk halves overlapping the transposes, to shorten the
            u -> v latency chain."""
            # Per-chunk copies alternate DVE/ACT so the transpose ->
            # y1n -> v chain pipelines at 128-column granularity.
            y1T = work.tile([P, N], bf16, tag="y1T")
            for c in (0, 2):
                nc.vector.tensor_scalar_mul(
                    y1T[:, 128 * c:128 * (c + 1)],
                    uT[:, 128 * c:128 * (c + 1)], 0.0625)
            for c in (1, 3):
                nc.scalar.activation(
                    out=y1T[:, 128 * c:128 * (c + 1)],
                    in_=uT[:, 128 * c:128 * (c + 1)],
                    func=Copy, scale=0.0625)
            acc = ps_a.tile([P, N], f32, tag="acc")
            ytp = ps_t.tile([P, C, P], bf16, tag="ytp")
            for c in range(C):
                nc.tensor.transpose(ytp[:, c, :], y1T[:, 128 * c:128 * (c + 1)], ident)
            # t1 before t0: both are ready once y1T lands, while t0's xt
            # may still be in flight on the slow SWDGE path
            for q in range(Q):
                sl = slice(32 * q, 32 * q + 32)
                nc.tensor.matmul(acc[sl, :], wept[sl, 1, :], y1T[sl, :],
                                 start=True, stop=False,
                                 tile_position=(32 * q, 32 * q))
            for q in range(Q):
                sl = slice(32 * q, 32 * q + 32)
                nc.tensor.matmul(acc[sl, :], wept[sl, 0, :], xt[sl, g, :],
                                 start=False, stop=False,
                                 tile_position=(32 * q, 32 * q))
            y1n = work.tile([P, C, Q, F], f8, tag="y1n")
            for c in range(C):
                src = ytp[:, c, :].rearrange("p (q f) -> p q f", q=Q)
                if c % 2 == 0:
                    nc.scalar.activation(out=y1n[:, c], in_=src, func=Copy)
                else:
                    nc.vector.tensor_copy(y1n[:, c], src)
            return y1T, y1n, acc

        def stage_v(g, y1n):
            """vT = 65536*v^T; 4 col groups stream the PE concurrently."""
            at = ats[g]
            vT = ps_v.tile([P, N], f32, tag="vT")
            for c in range(C):
                for q in range(Q):
                    nc.tensor.matmul(
                        vT[32 * q:32 * q + 32, :],
                        y1n[:, c, q, :],
                        at[:, c, q, :],
                        start=(c == 0), stop=(c == C - 1),
                        tile_position=(0, 32 * q),
                    )
            return vT

        def stage_vb(g, vT):
            """vb = vT/256 - issued right after stage_v so it is done by
            the time t2 leads the next wave's PE FIFO."""
            vb = work.tile([P, N], bf16, tag="vb")
            nc.vector.tensor_scalar_mul(vb, vT, 0.00390625)
            return vb

        def stage_t2(g, vb, acc):
            """t2 epilogue matmul (W2 term) - leads the PE FIFO of its
            wave since vb was computed an iteration earlier."""
            for q in range(Q):
                sl = slice(32 * q, 32 * q + 32)
                nc.tensor.matmul(acc[sl, :], wept[sl, 2, :], vb[sl, :],
                                 start=False, stop=True,
                                 tile_position=(32 * q, 32 * q))

        def stage_o(g, acc):
            """relu(+bias), residual, out-DMA by halves."""
            H = N // 2
            o4 = work.tile([P, N], bf16, tag="o4")
            if zero_bias:
                # out = max(acc, 0) + x, by halves: the first half's DMA
                # issue overlaps the second half's compute
                for h, eng in ((0, nc.scalar), (1, nc.sync)):
                    hs = slice(h * H, (h + 1) * H)
                    nc.vector.scalar_tensor_tensor(
                        out=o4[:, hs], in0=acc[:, hs], scalar=0.0,
                        in1=xt[:, g, hs], op0=mybir.AluOpType.max,
                        op1=mybir.AluOpType.add)
                    eng.dma_start(out=out_d[g, :, hs], in_=o4[:, hs])
            else:
                r4 = work.tile([P, N], bf16, tag="r4")
                nc.scalar.activation(
                    out=r4, in_=acc, func=Relu, bias=bcol, scale=1.0,
                )
                nc.vector.tensor_add(o4, r4, xt[:, g, :])
                nc.sync.dma_start(out=out_d[g], in_=o4)

        # Wave order keeps the PE FIFO dense: t2(i-2) first (its vb was
        # computed an iteration earlier), then m(i-1) (whose PE work is
        # gated only on the y1T copies), o4/outs of i-2, u(i) pair 1,
        # v(i-1) + its vb, then u(i) pair 2 - matching the ~3.3us DMA
        # pair cadence, with fillers bridging arrival jitter.
        st = {}
        for i in range(G + 2):
            if 0 <= i - 2 < G:
                g = i - 2
                stage_t2(g, st[g]["vb"], st[g]["acc"])
            if 0 <= i - 1 < G:
                g = i - 1
                y1T, y1n, acc = stage_m(g, st[g]["uT"])
                st[g]["acc"] = acc
                st[g]["y1n"] = y1n
            if 0 <= i - 2 < G:
                g = i - 2
                stage_o(g, st[g]["acc"])
                del st[g]
            if 0 <= i - 1 < G:
                g = i - 1
                if i >= 2:
                    fillers((0, 0, 4, 2, 0)[i])
                vT = stage_v(g, st[g]["y1n"])
                st[g]["vb"] = stage_vb(g, vT)
            if i < G:
                fillers((4, 12, 2, 0)[i])
                st[i] = {"uT": stage_u(i, 0)}
                if i >= 1:
                    fillers((0, 4, 2, 2)[i])
                stage_u(i, 1, st[i]["uT"])

    nc.finalize()
    return nc


def kernel(adj, x, W, b):
    import ml_dtypes

    adj = np.ascontiguousarray(adj, dtype=np.float32)
    x = np.ascontiguousarray(x, dtype=np.float32)
    W = np.asarray(W, dtype=np.float32)
    b = np.asarray(b, dtype=np.float32)

    f8np = ml_dtypes.float8_e4m3
    bfnp = ml_dtypes.bfloat16

    deg = adj.sum(-1)                                    # [B, N] exact f32
    dv16 = np.where(deg > 0, 16.0 / np.sqrt(deg), 0.0).astype(np.float32)

    # epilogue weights with the fp8 rescales folded in
    w0 = W[0] - W[2]
    w1 = (-W[1]) / 256.0
    w2 = (2.0 * W[2]) / 256.0
    wept = np.tile(np.stack([w0, w1, w2], axis=1), (4, 1, 1)).astype(bfnp)  # [128,3,32]
    ident = np.eye(P, dtype=np.float32).astype(bfnp)
    bcol = np.tile(b.reshape(1, F), (4, 1)).reshape(P, 1).astype(np.float32)

    zero_bias = bool(np.all(b == 0.0))
    key = ("nc", zero_bias)
    if key not in _cache:
        _cache[key] = build_nc(zero_bias)
    nc = _cache[key]

    in_maps = []
    for i in range(NCORES):
        sl = slice(i * S, (i + 1) * S)
        a = adj[sl]      # [16, 512, 512]
        xs = x[sl]       # [16, 512, 32]
        dv = dv16[sl]    # [16, 512] = 16*dinv

        # adjF = 256 * Ahat = (16 dinv) A (16 dinv), entries ~A in [0,1]
        ahat = a * dv[:, :, None] * dv[:, None, :]
        # adj8[p, g, c, q, n] = adjF_{4g+q}[n, 128c+p] (= adjF^T chunks)
        adj8 = np.ascontiguousarray(
            ahat.transpose(0, 2, 1).reshape(G, Q, C, P, N).transpose(3, 0, 2, 1, 4)
        ).astype(f8np)
        # xq8[p, g, c, q, f] = 16 * x[4g+q][128c+p, f]  (natural layout)
        xq8 = np.ascontiguousarray(
            (16.0 * xs).reshape(G, Q, C, P, F).transpose(3, 0, 2, 1, 4)
        ).astype(f8np)
        # xt[32q+f, g, n] = x[4g+q][n, f]^T
        xt4 = np.ascontiguousarray(
            xs.transpose(0, 2, 1).reshape(G, P, N).transpose(1, 0, 2)
        ).astype(bfnp)

        in_maps.append({
            "adj8": adj8,
            "xq8": xq8,
            "xt": xt4,
            "wept": wept,
            "ident": ident,
            "bcol": bcol,
        })

    trace = os.environ.get("KERNEL_TRACE") == "1"
    kw = {}
    if trace:
        _install_ntff_hook()
        import concourse.bass_utils as _bu
        _bu.upload_artifacts = lambda t: t  # no bucket in this container
        kw["tmpdir"] = os.environ.get("KERNEL_TRACE_DIR") or None
    res = run_bass_kernel_spmd(
        nc, in_maps, core_ids=list(range(NCORES)), trace=trace, **kw,
    )
    if trace and res.exec_time_ns is not None:
        print(f"HW exec time: {res.exec_time_ns} ns")

    # out[g, 32q+o, n] -> sample 4g+q, [n, o]
    outs = []
    for i in range(NCORES):
        og = np.asarray(res.results[i]["out"]).astype(np.float32)  # [G, 128, 512]
        outs.append(og.reshape(G, Q, F, N).transpose(0, 1, 3, 2).reshape(S, N, F))
    return np.ascontiguousarray(np.concatenate(outs, axis=0))


# revision 61
# speedup vs baseline: 1.1658x; 1.1658x over previous
"""ChebConv layer (B=128, N=512, F=32, K=3) on 8 TRN2 NeuronCores.

Math: with lambda_max = 2.0, Lhat = -Ahat, Ahat = S A S with S = diag(dinv).
Host folds S into the adjacency: ship adjF = 256*Ahat as fp8 (entries ~A,
good e4m3 range) and xq = 16*x as fp8. Then on-chip:
    uT  = (16x)^T (256Ahat)^T          = 4096*y1^T,  y1 = Ahat x  (T1 = -y1)
    y1T = uT/16 (bf16) = 256*y1; transpose -> y1n (fp8 natural layout)
    vT  = y1n^T (256Ahat)^T            = 65536*v^T,  v = Ahat^2 x
    out = relu( x(W0-W2) + (256y1)(-W1/256) + (256v)(2W2/256) + b ) + x
No per-node dinv scaling on-chip at all - it all lives in the host prep.

Sharding: data-parallel over batch, 16 samples/core as 4 groups of 4.
Per group of 4 samples (quadrant q = partition group 32q:32q+32):
  - u/v matmuls use fp8 DoubleRow perf mode: chunk pairs (c0,c1),(c2,c3)
    fused into one instruction each (0.5 cyc/row = 4x fewer PE column
    cycles vs one matmul per 128-chunk), 4 col groups concurrent via
    tile_position=(0,32q).
  - epilogue terms t0 (x*W0') and t1 (y1*W1') issue right after y1T so
    only the t2 (v) matmul trails the v-stage.
DMA: adj chunks spread over the three DGE paths (SP ring, ACT ring,
SWDGE) so each group's DoubleRow pair lands near-simultaneously; xq/xt
ride the ACT ring up front, outs ride the SP ring.
"""

import os
import sys

sys.path.insert(0, "/opt/trn_rl_repo")

import numpy as np

import concourse.bass as bass
from concourse import bacc
import concourse.mybir as mybir
import concourse.tile as tile
from concourse.bass_utils import run_bass_kernel_spmd
from contextlib import ExitStack

B, N, F = 128, 512, 32
NCORES = 8
S = B // NCORES          # samples per core (16)
P = 128                  # SBUF partitions
C = N // P               # m-chunks per sample (4)
Q = 4                    # samples per group (one per quadrant)
G = S // Q               # groups per core (4)

f32 = mybir.dt.float32
bf16 = mybir.dt.bfloat16
f8 = mybir.dt.float8e4

_cache = {}


def _install_ntff_hook():
    """Provide antenv.axon_hooks (missing in this image) so trace=True works."""
    import contextlib
    import ctypes
    import types

    try:
        from antenv.axon_hooks import get_axon_ntff_profile_hook  # noqa: F401
        return
    except ImportError:
        pass
    so_path = "/opt/axon/libaxon_pjrt.so"
    if not os.path.exists(so_path):
        return
    lib = ctypes.CDLL(so_path)
    if not hasattr(lib, "axon_start_nrt_profile"):
        return
    lib.axon_start_nrt_profile.argtypes = [
        ctypes.POINTER(ctypes.c_int64), ctypes.c_size_t,
    ]
    lib.axon_start_nrt_profile.restype = ctypes.c_int64
    lib.axon_stop_nrt_profile.argtypes = [ctypes.c_char_p]
    lib.axon_stop_nrt_profile.restype = ctypes.c_int64

    @contextlib.contextmanager
    def _hook(output_dir, device_ids):
        import jax

        jax.devices()
        if device_ids:
            ids = (ctypes.c_int64 * len(device_ids))(*device_ids)
            rc = lib.axon_start_nrt_profile(ids, len(device_ids))
        else:
            rc = lib.axon_start_nrt_profile(None, 0)
        if rc != 0:
            raise RuntimeError(f"axon_start_nrt_profile rc={rc}")
        try:
            yield
        finally:
            n = lib.axon_stop_nrt_profile(str(output_dir).encode())
            print(f"profile: {n} file(s) written to {output_dir}", file=sys.stderr)

    mod = types.ModuleType("antenv.axon_hooks")
    state = {"hook": _hook}
    mod.get_axon_ntff_profile_hook = lambda: state["hook"]
    mod.set_axon_ntff_profile_hook = lambda h: state.update(hook=h)
    sys.modules["antenv.axon_hooks"] = mod


def build_nc(zero_bias):
    nc = bacc.Bacc()
    adj_d = nc.declare_dram_parameter("adj8", [P, G, C, Q, N], f8, isOutput=False)
    xq_d = nc.declare_dram_parameter("xq8", [P, G, C, Q, F], f8, isOutput=False)
    xt_d = nc.declare_dram_parameter("xt", [P, G, N], bf16, isOutput=False)
    wept_d = nc.declare_dram_parameter("wept", [P, 3, F], bf16, isOutput=False)
    id_d = nc.declare_dram_parameter("ident", [P, P], bf16, isOutput=False)
    b_d = nc.declare_dram_parameter("bcol", [P, 1], f32, isOutput=False)
    out_d = nc.declare_dram_parameter("out", [G, P, N], bf16, isOutput=True)

    Copy = mybir.ActivationFunctionType.Copy
    Relu = mybir.ActivationFunctionType.Relu

    with tile.TileContext(nc) as tc, ExitStack() as ctx:
        consts = ctx.enter_context(tc.tile_pool(name="consts", bufs=1))
        adj_pool = ctx.enter_context(tc.tile_pool(name="adj", bufs=G))
        work = ctx.enter_context(tc.tile_pool(name="work", bufs=3))
        ps_u = ctx.enter_context(tc.tile_pool(name="psu", bufs=1, space="PSUM"))
        ps_v = ctx.enter_context(tc.tile_pool(name="psv", bufs=1, space="PSUM"))
        ps_a = ctx.enter_context(tc.tile_pool(name="psa", bufs=2, space="PSUM"))
        ps_t = ctx.enter_context(tc.tile_pool(name="pst", bufs=2, space="PSUM"))
        ps_w = ctx.enter_context(tc.tile_pool(name="psw", bufs=1, space="PSUM"))

        ats = [adj_pool.tile([P, C, Q, N], f8, tag=f"at{g}", name=f"at{g}")
               for g in range(G)]
        xq = consts.tile([P, G, C, Q, F], f8, tag="xq")
        xt = consts.tile([P, G, N], bf16, tag="xt")
        ident = consts.tile([P, P], bf16, tag="ident")
        wept = consts.tile([P, 3, F], bf16, tag="wept")
        bcol = consts.tile([P, 1], f32, tag="bcol")

        # PE warm-up: dependency-free matmuls bridge from engine start to
        # the first adj pair arrival so HAM hits K=8/8 as real work begins
        # (a cold PE runs at reduced duty and falls behind the stream).
        # PE warm-up: dependency-free matmuls bridge from engine start
        # (~7us) to the first adj pair arrival (~13us). HAM holds the PE
        # at full speed (k=8) only while it is continuously busy - an
        # idle gap triggers a ~10us half-speed (k=4) penalty - so the
        # warm-up must hand off seamlessly to real work.
        warm = consts.tile([P, N], bf16, tag="warm")
        nc.vector.memset(warm, 0.0)
        wps = ps_w.tile([F, N], f32, tag="wps")
        for _ in range(24):
            nc.tensor.matmul(wps, warm[:, 0:F], warm, start=True, stop=True)

        def fillers(n):
            """Junk matmuls that keep the PE continuously busy across an
            expected DMA-arrival gap (an idle PE drops to HAM k=4)."""
            for _ in range(n):
                nc.tensor.matmul(wps[:, 0:128], warm[:, 0:F], warm[:, 0:128],
                                 start=True, stop=True)

        # --- input DMAs, 512KB chunk-pairs. Per group, (c0,c1) rides the
        # SP ring and (c2,c3) the ACT ring in lockstep (~3.3us cadence);
        # xq leads the SP ring so u(g0) can start on first-pair arrival.
        # Consts + xt ride the SWDGE (latency-insensitive). Outs join the
        # SP ring from stage_e.
        nc.sync.dma_start(out=xq, in_=xq_d[:, :, :, :, :])
        nc.gpsimd.dma_start(out=ident, in_=id_d[:, :])
        nc.gpsimd.dma_start(out=xt[:, 0:2, :], in_=xt_d[:, 0:2, :])
        nc.gpsimd.dma_start(out=wept, in_=wept_d[:, :, :])
        nc.gpsimd.dma_start(out=bcol, in_=b_d[:, :])
        nc.scalar.dma_start(out=ats[0][:, 2:4], in_=adj_d[:, 0, 2:4])
        nc.sync.dma_start(out=ats[0][:, 0:2], in_=adj_d[:, 0, 0:2])
        nc.scalar.dma_start(out=ats[1][:, 2:4], in_=adj_d[:, 1, 2:4])
        nc.sync.dma_start(out=ats[1][:, 0:2], in_=adj_d[:, 1, 0:2])
        nc.gpsimd.dma_start(out=ats[2][:, 2:4], in_=adj_d[:, 2, 2:4])
        nc.sync.dma_start(out=ats[2][:, 0:2], in_=adj_d[:, 2, 0:2])
        nc.scalar.dma_start(out=ats[3][:, 2:4], in_=adj_d[:, 3, 2:4])
        nc.sync.dma_start(out=ats[3][:, 0:2], in_=adj_d[:, 3, 0:2])
        nc.gpsimd.dma_start(out=xt[:, 2:4, :], in_=xt_d[:, 2:4, :])

        def stage_u(g, half, uT=None):
            """uT = 4096*y1^T; 4 col groups stream the PE concurrently.
            Issued as two chunk-pair halves matching DMA arrival."""
            at = ats[g]
            if uT is None:
                uT = ps_u.tile([P, N], f32, tag="uT")
            for c in (0, 1) if half == 0 else (2, 3):
                for q in range(Q):
                    nc.tensor.matmul(
                        uT[32 * q:32 * q + 32, :],
                        xq[:, g, c, q, :],
                        at[:, c, q, :],
                        start=(c == 0), stop=(c == C - 1),
                        tile_position=(0, 32 * q),
                    )
            return uT

        def stage_m(g, uT):
            """y1T = uT/16 (bf16), transpose to y1n fp8; early epilogue
            terms t0 (x) and t1 (y1) into acc. y1T is computed as two
            column halves on DVE and ACT in parallel, and y1n as two
            chunk halves overlapping the transposes, to shorten the
            u -> v latency chain."""
            # Per-chunk copies alternate DVE/ACT so the transpose ->
            # y1n -> v chain pipelines at 128-column granularity.
            y1T = work.tile([P, N], bf16, tag="y1T")
            for c in (0, 2):
                nc.vector.tensor_scalar_mul(
                    y1T[:, 128 * c:128 * (c + 1)],
                    uT[:, 128 * c:128 * (c + 1)], 0.0625)
            for c in (1, 3):
                nc.scalar.activation(
                    out=y1T[:, 128 * c:128 * (c + 1)],
                    in_=uT[:, 128 * c:128 * (c + 1)],
                    func=Copy, scale=0.0625)
            acc = ps_a.tile([P, N], f32, tag="acc")
            ytp = ps_t.tile([P, C, P], bf16, tag="ytp")
            for c in range(C):
                nc.tensor.transpose(ytp[:, c, :], y1T[:, 128 * c:128 * (c + 1)], ident)
            # t1 before t0: both are ready once y1T lands, while t0's xt
            # may still be in flight on the slow SWDGE path
            for q in range(Q):
                sl = slice(32 * q, 32 * q + 32)
                nc.tensor.matmul(acc[sl, :], wept[sl, 1, :], y1T[sl, :],
                                 start=True, stop=False,
                                 tile_position=(32 * q, 32 * q))
            for q in range(Q):
                sl = slice(32 * q, 32 * q + 32)
                nc.tensor.matmul(acc[sl, :], wept[sl, 0, :], xt[sl, g, :],
                                 start=False, stop=False,
                                 tile_position=(32 * q, 32 * q))
            y1n = work.tile([P, C, Q, F], f8, tag="y1n")
            for c in range(C):
                src = ytp[:, c, :].rearrange("p (q f) -> p q f", q=Q)
                if c % 2 == 0:
                    nc.scalar.activation(out=y1n[:, c], in_=src, func=Copy)
                else:
                    nc.vector.tensor_copy(y1n[:, c], src)
            return y1T, y1n, acc

        def stage_v(g, y1n):
            """vT = 65536*v^T; 4 col groups stream the PE concurrently."""
            at = ats[g]
            vT = ps_v.tile([P, N], f32, tag="vT")
            for c in range(C):
                for q in range(Q):
                    nc.tensor.matmul(
                        vT[32 * q:32 * q + 32, :],
                        y1n[:, c, q, :],
                        at[:, c, q, :],
                        start=(c == 0), stop=(c == C - 1),
                        tile_position=(0, 32 * q),
                    )
            return vT

        def stage_vb(g, vT):
            """vb = vT/256 - issued right after stage_v so it is done by
            the time t2 leads the next wave's PE FIFO."""
            vb = work.tile([P, N], bf16, tag="vb")
            nc.vector.tensor_scalar_mul(vb, vT, 0.00390625)
            return vb

        def stage_t2(g, vb, acc):
            """t2 epilogue matmul (W2 term) - leads the PE FIFO of its
            wave since vb was computed an iteration earlier."""
            for q in range(Q):
                sl = slice(32 * q, 32 * q + 32)
                nc.tensor.matmul(acc[sl, :], wept[sl, 2, :], vb[sl, :],
                                 start=False, stop=True,
                                 tile_position=(32 * q, 32 * q))

        def stage_o(g, acc):
            """relu(+bias), residual, out-DMA by halves."""
            H = N // 2
            o4 = work.tile([P, N], bf16, tag="o4")
            if zero_bias:
                # out = max(acc, 0) + x, by halves: the first half's DMA
                # issue overlaps the second half's compute
                for h, eng in ((0, nc.scalar), (1, nc.sync)):
                    hs = slice(h * H, (h + 1) * H)
                    nc.vector.scalar_tensor_tensor(
                        out=o4[:, hs], in0=acc[:, hs], scalar=0.0,
                        in1=xt[:, g, hs], op0=mybir.AluOpType.max,
                        op1=mybir.AluOpType.add)
                    eng.dma_start(out=out_d[g, :, hs], in_=o4[:, hs])
            else:
                r4 = work.tile([P, N], bf16, tag="r4")
                nc.scalar.activation(
                    out=r4, in_=acc, func=Relu, bias=bcol, scale=1.0,
                )
                nc.vector.tensor_add(o4, r4, xt[:, g, :])
                nc.sync.dma_start(out=out_d[g], in_=o4)

        # Wave order keeps the PE FIFO dense: t2(i-2) first (its vb was
        # computed an iteration earlier), then m(i-1) (whose PE work is
        # gated only on the y1T copies), o4/outs of i-2, u(i) pair 1,
        # v(i-1) + its vb, then u(i) pair 2 - matching the ~3.3us DMA
        # pair cadence, with fillers bridging arrival jitter.
        st = {}
        for i in range(G + 2):
            if 0 <= i - 2 < G:
                g = i - 2
                stage_t2(g, st[g]["vb"], st[g]["acc"])
            if 0 <= i - 1 < G:
                g = i - 1
                y1T, y1n, acc = stage_m(g, st[g]["uT"])
                st[g]["acc"] = acc
                st[g]["y1n"] = y1n
            if 0 <= i - 2 < G:
                g = i - 2
                stage_o(g, st[g]["acc"])
                del st[g]
            if 0 <= i - 1 < G:
                g = i - 1
                if i >= 2:
                    fillers((0, 0, 4, 2, 0)[i])
                vT = stage_v(g, st[g]["y1n"])
                st[g]["vb"] = stage_vb(g, vT)
            if i < G:
                if i >= 1:
                    fillers((0, 12, 2, 0)[i])
                st[i] = {"uT": stage_u(i, 0)}
                if i >= 1:
                    fillers((0, 4, 2, 2)[i])
                stage_u(i, 1, st[i]["uT"])

    nc.finalize()
    return nc


def kernel(adj, x, W, b):
    import ml_dtypes

    adj = np.ascontiguousarray(adj, dtype=np.float32)
    x = np.ascontiguousarray(x, dtype=np.float32)
    W = np.asarray(W, dtype=np.float32)
    b = np.asarray(b, dtype=np.float32)

    f8np = ml_dtypes.float8_e4m3
    bfnp = ml_dtypes.bfloat16

    deg = adj.sum(-1)                                    # [B, N] exact f32
    dv16 = np.where(deg > 0, 16.0 / np.sqrt(deg), 0.0).astype(np.float32)

    # epilogue weights with the fp8 rescales folded in
    w0 = W[0] - W[2]
    w1 = (-W[1]) / 256.0
    w2 = (2.0 * W[2]) / 256.0
    wept = np.tile(np.stack([w0, w1, w2], axis=1), (4, 1, 1)).astype(bfnp)  # [128,3,32]
    ident = np.eye(P, dtype=np.float32).astype(bfnp)
    bcol = np.tile(b.reshape(1, F), (4, 1)).reshape(P, 1).astype(np.float32)

    zero_bias = bool(np.all(b == 0.0))
    key = ("nc", zero_bias)
    if key not in _cache:
        _cache[key] = build_nc(zero_bias)
    nc = _cache[key]

    in_maps = []
    for i in range(NCORES):
        sl = slice(i * S, (i + 1) * S)
        a = adj[sl]      # [16, 512, 512]
        xs = x[sl]       # [16, 512, 32]
        dv = dv16[sl]    # [16, 512] = 16*dinv

        # adjF = 256 * Ahat = (16 dinv) A (16 dinv), entries ~A in [0,1]
        ahat = a * dv[:, :, None] * dv[:, None, :]
        # adj8[p, g, c, q, n] = adjF_{4g+q}[n, 128c+p] (= adjF^T chunks)
        adj8 = np.ascontiguousarray(
            ahat.transpose(0, 2, 1).reshape(G, Q, C, P, N).transpose(3, 0, 2, 1, 4)
        ).astype(f8np)
        # xq8[p, g, c, q, f] = 16 * x[4g+q][128c+p, f]  (natural layout)
        xq8 = np.ascontiguousarray(
            (16.0 * xs).reshape(G, Q, C, P, F).transpose(3, 0, 2, 1, 4)
        ).astype(f8np)
        # xt[32q+f, g, n] = x[4g+q][n, f]^T
        xt4 = np.ascontiguousarray(
            xs.transpose(0, 2, 1).reshape(G, P, N).transpose(1, 0, 2)
        ).astype(bfnp)

        in_maps.append({
            "adj8": adj8,
            "xq8": xq8,
            "xt": xt4,
            "wept": wept,
            "ident": ident,
            "bcol": bcol,
        })

    trace = os.environ.get("KERNEL_TRACE") == "1"
    kw = {}
    if trace:
        _install_ntff_hook()
        import concourse.bass_utils as _bu
        _bu.upload_artifacts = lambda t: t  # no bucket in this container
        kw["tmpdir"] = os.environ.get("KERNEL_TRACE_DIR") or None
    res = run_bass_kernel_spmd(
        nc, in_maps, core_ids=list(range(NCORES)), trace=trace, **kw,
    )
    if trace and res.exec_time_ns is not None:
        print(f"HW exec time: {res.exec_time_ns} ns")

    # out[g, 32q+o, n] -> sample 4g+q, [n, o]
    outs = []
    for i in range(NCORES):
        og = np.asarray(res.results[i]["out"]).astype(np.float32)  # [G, 128, 512]
        outs.append(og.reshape(G, Q, F, N).transpose(0, 1, 3, 2).reshape(S, N, F))
    return np.ascontiguousarray(np.concatenate(outs, axis=0))
